# revision 5
# baseline (speedup 1.0000x reference)
"""CoAttention kernel for Trainium2 (8 NeuronCores, pure data parallel).

Problem shapes (hardcoded, from the reference nn.Module):
  B=32, E=8, LC=256, LE=512, D=256, OUT=128, H=1
  claim:    (B, E, LC, D) f32      claim_len_mask:    (B, E, 1, LC) i32
  evidence: (B, E, LE, D) f32      evidence_len_mask: (B, E, 1, LE) i32
  W1, W2: (2D, OUT)   w1, w2: (OUT, H)
  returns (c_hat, e_hat), each (B, E, 1, H*D) f32

Math per (b, e) pair:
  c_mean = mean(claim, L)                       # (1, D)
  h_e  = tanh(evidence @ W1[:D] + c_mean @ W1[D:])   # (LE, OUT)
  a_e  = softmax(mask(h_e @ w2))                # (LE, 1), softmax over LE
  e_hat = a_e^T @ evidence                      # (1, D)
  ... symmetric for claim with W2/w1 and e_mean.

Device strategy per pair (one NeuronCore handles 32 pairs = 4 batches x 8 evi):
  - Host supplies both natural (L-part) and transposed (D-part) layouts of
    claim/evidence; the PE contracts the partition dim and each big tensor is
    contracted along both of its axes.
  - sums over L via DVE tensor_scalar(accum_out) on transposed tiles
  - pooled projection: tiny PE matmuls, 1/L folded into W*[D:] on host
  - hT (OUT, L) = W_a^T @ seqT on PE; tanh + pooled-bias fused in one ACT op
  - att chunks (128, 1) = hT_chunk^T @ w on PE (partition layout)
  - mask add (att + (mask-1)*1e30) on DVE; exp on ACT (one op, both branches)
  - unnormalized output (1, D) = sum_k u_k^T @ seq_chunk on PE; denominator
    sum(u) via ones^T @ u_k matmuls
  - normalize during PSUM evacuation: ACT copy with scale = 1/denom
"""

import numpy as np

import concourse.bass as bass
import concourse.mybir as mybir
import concourse.tile as tile
from concourse import bacc
from concourse.bass_utils import run_bass_kernel_spmd

B, E, LC, LE, D, OUT = 32, 8, 256, 512, 256, 128
NCORES = 8
BPC = B // NCORES          # batches per core
P32 = BPC * E              # pairs per core = 32
NEGBIG = 1.0e30

f32 = mybir.dt.float32
AF = mybir.ActivationFunctionType
ALU = mybir.AluOpType

_BUILT = {}


def _emit(tc, dram):
    nc = tc.nc
    ev_d = dram["ev"].ap()       # (P32, LE, D)
    evT_d = dram["evT"].ap()     # (P32, D, LE)
    cl_d = dram["cl"].ap()       # (P32, LC, D)
    clT_d = dram["clT"].ap()     # (P32, D, LC)
    c_hat_d = dram["c_hat"].ap()  # (P32, D)
    e_hat_d = dram["e_hat"].ap()  # (P32, D)

    with (
        tc.tile_pool(name="const", bufs=1) as const,
        tc.tile_pool(name="inp", bufs=3) as inp,
        tc.tile_pool(name="hts", bufs=2) as hts,
        tc.tile_pool(name="sml", bufs=3) as sml,
        tc.tile_pool(name="scr", bufs=2) as scr,
        tc.tile_pool(name="ph", bufs=2, space="PSUM") as ph,
        tc.tile_pool(name="pa", bufs=2, space="PSUM") as pa,
        tc.tile_pool(name="po", bufs=2, space="PSUM") as po,
    ):
        # ---- constants ----
        w1a_t = const.tile([128, 2, 128], f32)   # W1[:D] chunked (j, k, o)
        nc.sync.dma_start(w1a_t[:], dram["w1a"].ap())
        w1bs_t = const.tile([128, 2, 128], f32)  # W1[D:]/LC
        nc.sync.dma_start(w1bs_t[:], dram["w1bs"].ap())
        w2a_t = const.tile([128, 2, 128], f32)   # W2[:D]
        nc.sync.dma_start(w2a_t[:], dram["w2a"].ap())
        w2bs_t = const.tile([128, 2, 128], f32)  # W2[D:]/LE
        nc.sync.dma_start(w2bs_t[:], dram["w2bs"].ap())
        w2v_t = const.tile([128, 1], f32)        # w2 (evidence branch head)
        nc.sync.dma_start(w2v_t[:], dram["w2v"].ap())
        w1v_t = const.tile([128, 1], f32)        # w1 (claim branch head)
        nc.sync.dma_start(w1v_t[:], dram["w1v"].ap())
        mne_t = const.tile([128, P32, 4], f32)   # (ev mask - 1)*1e30, (j, p, k)
        nc.sync.dma_start(mne_t[:], dram["mne"].ap())
        mnc_t = const.tile([128, P32, 2], f32)
        nc.sync.dma_start(mnc_t[:], dram["mnc"].ap())
        ones_t = const.tile([128, 1], f32)
        nc.vector.memset(ones_t[:], 1.0)

        for p in range(P32):
            # ---- loads ----
            evT_t = inp.tile([128, 2, LE], f32, tag="evT")
            nc.sync.dma_start(evT_t[:], evT_d[p].rearrange("(k j) l -> j k l", j=128))
            ev_t = inp.tile([128, 4, D], f32, tag="ev")
            nc.sync.dma_start(ev_t[:], ev_d[p].rearrange("(c j) d -> j c d", j=128))
            clT_t = inp.tile([128, 2, LC], f32, tag="clT")
            nc.sync.dma_start(clT_t[:], clT_d[p].rearrange("(k j) l -> j k l", j=128))
            cl_t = inp.tile([128, 2, D], f32, tag="cl")
            nc.sync.dma_start(cl_t[:], cl_d[p].rearrange("(c j) d -> j c d", j=128))

            # ---- column sums over L (transposed layout, DVE fused accum) ----
            # meanT cols: 0-1 = sum(claim) D-chunks, 2-3 = sum(evidence)
            meanT = sml.tile([128, 4], f32, tag="meanT")
            scr_cl = scr.tile([128, 2, LC], f32, tag="scr_cl")
            for k in range(2):
                nc.vector.tensor_scalar(
                    scr_cl[:, k, :], clT_t[:, k, :], 1.0, 0.0, ALU.mult, ALU.add,
                    accum_out=meanT[:, k : k + 1],
                )
            scr_ev = scr.tile([128, 2, LE], f32, tag="scr_ev")
            for k in range(2):
                nc.vector.tensor_scalar(
                    scr_ev[:, k, :], evT_t[:, k, :], 1.0, 0.0, ALU.mult, ALU.add,
                    accum_out=meanT[:, 2 + k : 3 + k],
                )

            # ---- pooled projections (pb = mean @ W[D:]) ----
            # patt cols: 0-3 att_ev, 4-5 att_cl, 6 pb_ev, 7 pb_cl, 8 den_ev, 9 den_cl
            patt = pa.tile([128, 10], f32, tag="patt")
            for k in range(2):
                nc.tensor.matmul(
                    patt[:, 6:7], w1bs_t[:, k, :], meanT[:, k : k + 1],
                    start=(k == 0), stop=(k == 1),
                )
            for k in range(2):
                nc.tensor.matmul(
                    patt[:, 7:8], w2bs_t[:, k, :], meanT[:, 2 + k : 3 + k],
                    start=(k == 0), stop=(k == 1),
                )
            pb_sb = sml.tile([128, 2], f32, tag="pb_sb")
            nc.vector.tensor_copy(pb_sb[:], patt[:, 6:8])

            # ---- hT = W_a^T @ seqT ----
            ht_ev_ps = ph.tile([128, LE], f32, tag="ht_ev_ps")
            for k in range(2):
                nc.tensor.matmul(
                    ht_ev_ps[:], w1a_t[:, k, :], evT_t[:, k, :],
                    start=(k == 0), stop=(k == 1),
                )
            ht_cl_ps = ph.tile([128, LC], f32, tag="ht_cl_ps")
            for k in range(2):
                nc.tensor.matmul(
                    ht_cl_ps[:], w2a_t[:, k, :], clT_t[:, k, :],
                    start=(k == 0), stop=(k == 1),
                )
            # tanh with fused pooled-projection bias (per-partition = per-OUT)
            ht_ev = hts.tile([128, LE], f32, tag="ht_ev")
            nc.scalar.activation(ht_ev[:], ht_ev_ps[:], AF.Tanh, bias=pb_sb[:, 0:1])
            ht_cl = hts.tile([128, LC], f32, tag="ht_cl")
            nc.scalar.activation(ht_cl[:], ht_cl_ps[:], AF.Tanh, bias=pb_sb[:, 1:2])

            # ---- attention logits, partition layout (128, 1) per L-chunk ----
            for k in range(4):
                nc.tensor.matmul(
                    patt[:, k : k + 1], ht_ev[:, 128 * k : 128 * (k + 1)], w2v_t[:],
                    start=True, stop=True,
                )
            for k in range(2):
                nc.tensor.matmul(
                    patt[:, 4 + k : 5 + k], ht_cl[:, 128 * k : 128 * (k + 1)], w1v_t[:],
                    start=True, stop=True,
                )

            # ---- mask add + exp (u = exp(att + (mask-1)*1e30)) ----
            u_t = sml.tile([128, 6], f32, tag="u_t")
            nc.vector.tensor_add(u_t[:, 0:4], patt[:, 0:4], mne_t[:, p, :])
            nc.vector.tensor_add(u_t[:, 4:6], patt[:, 4:6], mnc_t[:, p, :])
            ue_t = sml.tile([128, 6], f32, tag="ue_t")
            nc.scalar.activation(ue_t[:], u_t[:], AF.Exp)

            # ---- denominators: sum(u) via ones^T @ u_k ----
            for k in range(4):
                nc.tensor.matmul(
                    patt[0:1, 8:9], ones_t[:], ue_t[:, k : k + 1],
                    start=(k == 0), stop=(k == 3),
                )
            for k in range(2):
                nc.tensor.matmul(
                    patt[0:1, 9:10], ones_t[:], ue_t[:, 4 + k : 5 + k],
                    start=(k == 0), stop=(k == 1),
                )

            # ---- unnormalized outputs: sum_k u_k^T @ seq_chunk ----
            o_t = po.tile([1, 512], f32, tag="o_t")  # [0:256] ev out, [256:512] cl out
            for k in range(4):
                nc.tensor.matmul(
                    o_t[:, 0:D], ue_t[:, k : k + 1], ev_t[:, k, :],
                    start=(k == 0), stop=(k == 3),
                )
            for k in range(2):
                nc.tensor.matmul(
                    o_t[:, D : 2 * D], ue_t[:, 4 + k : 5 + k], cl_t[:, k, :],
                    start=(k == 0), stop=(k == 1),
                )

            # ---- normalize during evacuation, then store ----
            rec_t = sml.tile([1, 2], f32, tag="rec_t")
            nc.vector.reciprocal(rec_t[:, 0:1], patt[0:1, 8:9])
            nc.vector.reciprocal(rec_t[:, 1:2], patt[0:1, 9:10])
            res_ev = sml.tile([1, D], f32, tag="res_ev")
            nc.scalar.activation(res_ev[:], o_t[:, 0:D], AF.Copy, scale=rec_t[:, 0:1])
            res_cl = sml.tile([1, D], f32, tag="res_cl")
            nc.scalar.activation(res_cl[:], o_t[:, D : 2 * D], AF.Copy, scale=rec_t[:, 1:2])
            nc.sync.dma_start(e_hat_d[p : p + 1, :], res_ev[:])
            nc.sync.dma_start(c_hat_d[p : p + 1, :], res_cl[:])


def build():
    if "nc" in _BUILT:
        return _BUILT["nc"]
    nc = bacc.Bacc("TRN2")
    dram = {}
    dram["ev"] = nc.dram_tensor("ev", (P32, LE, D), f32, kind="ExternalInput")
    dram["evT"] = nc.dram_tensor("evT", (P32, D, LE), f32, kind="ExternalInput")
    dram["cl"] = nc.dram_tensor("cl", (P32, LC, D), f32, kind="ExternalInput")
    dram["clT"] = nc.dram_tensor("clT", (P32, D, LC), f32, kind="ExternalInput")
    dram["w1a"] = nc.dram_tensor("w1a", (128, 2, 128), f32, kind="ExternalInput")
    dram["w1bs"] = nc.dram_tensor("w1bs", (128, 2, 128), f32, kind="ExternalInput")
    dram["w2a"] = nc.dram_tensor("w2a", (128, 2, 128), f32, kind="ExternalInput")
    dram["w2bs"] = nc.dram_tensor("w2bs", (128, 2, 128), f32, kind="ExternalInput")
    dram["w2v"] = nc.dram_tensor("w2v", (128, 1), f32, kind="ExternalInput")
    dram["w1v"] = nc.dram_tensor("w1v", (128, 1), f32, kind="ExternalInput")
    dram["mne"] = nc.dram_tensor("mne", (128, P32, 4), f32, kind="ExternalInput")
    dram["mnc"] = nc.dram_tensor("mnc", (128, P32, 2), f32, kind="ExternalInput")
    dram["c_hat"] = nc.dram_tensor("c_hat", (P32, D), f32, kind="ExternalOutput")
    dram["e_hat"] = nc.dram_tensor("e_hat", (P32, D), f32, kind="ExternalOutput")

    with tile.TileContext(nc) as tc:
        _emit(tc, dram)
    nc.compile()
    _BUILT["nc"] = nc
    return nc


def _chunk_w(w):
    # (256, 128) -> (128, 2, 128): [j, k, o] = w[k*128 + j, o]
    return np.ascontiguousarray(w.reshape(2, 128, 128).transpose(1, 0, 2))


def _mask_neg_T(m, nchunk):
    # (P32, L) {0,1} -> (128, P32, nchunk): [j, p, k] = (m[p, k*128+j]-1)*1e30
    mn = (m.astype(np.float32) - 1.0) * NEGBIG
    return np.ascontiguousarray(mn.reshape(P32, nchunk, 128).transpose(2, 0, 1))


def make_in_maps(claim, claim_len_mask, evidence, evidence_len_mask, W1, w2, W2, w1):
    claim = np.asarray(claim, np.float32)
    evidence = np.asarray(evidence, np.float32)
    W1 = np.asarray(W1, np.float32)
    W2 = np.asarray(W2, np.float32)
    w1 = np.asarray(w1, np.float32)
    w2 = np.asarray(w2, np.float32)
    wmap = {
        "w1a": _chunk_w(W1[:D]),
        "w1bs": _chunk_w(W1[D:] * np.float32(1.0 / LC)),
        "w2a": _chunk_w(W2[:D]),
        "w2bs": _chunk_w(W2[D:] * np.float32(1.0 / LE)),
        "w2v": np.ascontiguousarray(w2.reshape(128, 1)),
        "w1v": np.ascontiguousarray(w1.reshape(128, 1)),
    }
    in_maps = []
    for c in range(NCORES):
        b0, b1 = c * BPC, (c + 1) * BPC
        ev = evidence[b0:b1].reshape(P32, LE, D)
        cl = claim[b0:b1].reshape(P32, LC, D)
        m = dict(wmap)
        m["ev"] = np.ascontiguousarray(ev)
        m["evT"] = np.ascontiguousarray(ev.transpose(0, 2, 1))
        m["cl"] = np.ascontiguousarray(cl)
        m["clT"] = np.ascontiguousarray(cl.transpose(0, 2, 1))
        m["mne"] = _mask_neg_T(
            np.asarray(evidence_len_mask[b0:b1]).reshape(P32, LE), 4)
        m["mnc"] = _mask_neg_T(
            np.asarray(claim_len_mask[b0:b1]).reshape(P32, LC), 2)
        in_maps.append(m)
    return in_maps


def assemble(results):
    c_hat = np.empty((B, E, 1, D), np.float32)
    e_hat = np.empty((B, E, 1, D), np.float32)
    for c, r in enumerate(results):
        b0, b1 = c * BPC, (c + 1) * BPC
        c_hat[b0:b1] = np.asarray(r["c_hat"]).reshape(BPC, E, 1, D)
        e_hat[b0:b1] = np.asarray(r["e_hat"]).reshape(BPC, E, 1, D)
    return c_hat, e_hat


def run(in_maps, **kwargs):
    nc = build()
    return run_bass_kernel_spmd(nc, in_maps, core_ids=list(range(NCORES)), **kwargs)


def kernel(**inputs):
    in_maps = make_in_maps(**inputs)
    res = run(in_maps)
    return assemble(res.results)
